# revision 1
# baseline (speedup 1.0000x reference)
# Bidirectional cross-attention Trainium2 kernel (Bass/Tile), 8-core head-parallel.
#
# Sharding: 16 heads / 8 cores = 2 heads per core (tensor parallel on h); each
# core computes its heads' projections, similarity, both softmax directions and
# its row-parallel partial of the final projections; host sums partials + bias.
#
# Design:
#  - everything 2-byte fp16 (data ranges are small, fp16 ~16x more precise
#    than bf16 at identical PE/DMA cost)
#  - exp computed ONCE per head (E stored fp16), [128,1024] psum tiles
#  - E^T via DMA xbar transposes emitted as one contiguous block per j-half
#    so they fire back-to-back (interleaved emission makes each call ~10x
#    slower on HW; back-to-back they run at full xbar rate)
#  - softmax sums ride along as a ones-column in the V operands
#  - normalization: DVE reciprocal (f32r) + K=1 PE ones-broadcast + DVE mul
#  - software-pipelined emission: H/G/norm/final work is sprinkled between
#    sim+exp iterations so PE work rides under the ACT-bound exp stream

import os
import sys

for _p in ("/opt/trn_rl_repo", "/root/.axon_site/_ro/trn_rl_repo"):
    if os.path.isdir(_p) and _p not in sys.path:
        sys.path.insert(0, _p)

import numpy as np

SEQ_MODE = os.environ.get("KSEQ", "0") == "1"

HEADS = 16
DIM_HEAD = 64
DIM = 1024
SEQ = 2048
N_CORES = 8
HPC = HEADS // N_CORES          # heads per core = 2
FPC = HPC * DIM_HEAD            # feature cols per core = 128
SCALE = DIM_HEAD ** -0.5


def _ts(i, size):
    return slice(i * size, (i + 1) * size)


def build_bass(seq=SEQ, dim=DIM, fpc=FPC, hpc=HPC, num_devices=N_CORES, stage='full'):
    import concourse.bacc as bacc
    import concourse.tile as tile
    import concourse.mybir as mybir
    from contextlib import ExitStack

    f32 = mybir.dt.float32
    f16 = mybir.dt.float16
    Exp = mybir.ActivationFunctionType.Exp

    P = 128
    KT = dim // P              # contraction tiles over DIM (8)
    NT = seq // P              # 128-blocks along sequence (16)
    NCH = seq // 512           # 512-chunks along sequence (4)
    J2 = min(1024, seq)        # exp-tile width
    N2CH = seq // J2           # exp-tile chunks along sequence
    HPT = J2 // 512            # 512-halves per exp tile
    ITPC = NT // NCH           # i-tiles per 512-chunk (4)
    OCH = dim // 512           # 512-chunks of output dim (2)
    dh = DIM_HEAD
    vw = dh + 1

    nc = bacc.Bacc("TRN2", target_bir_lowering=False, debug=False,
                   num_devices=num_devices)

    xT = nc.dram_tensor("xT", (dim, seq), f16, kind="ExternalInput").ap()
    cT = nc.dram_tensor("cT", (dim, seq), f16, kind="ExternalInput").ap()
    wqk = nc.dram_tensor("wqk", (dim, fpc), f16, kind="ExternalInput").ap()
    wv = nc.dram_tensor("wv", (dim, fpc), f16, kind="ExternalInput").ap()
    wcqk = nc.dram_tensor("wcqk", (dim, fpc), f16, kind="ExternalInput").ap()
    wcv = nc.dram_tensor("wcv", (dim, fpc), f16, kind="ExternalInput").ap()
    wout = nc.dram_tensor("wout", (fpc, dim), f16, kind="ExternalInput").ap()
    wcout = nc.dram_tensor("wcout", (fpc, dim), f16, kind="ExternalInput").ap()
    out_p = nc.dram_tensor("out_p", (seq, dim), f16, kind="ExternalOutput").ap()
    ctx_p = nc.dram_tensor("ctx_p", (seq, dim), f16, kind="ExternalOutput").ap()

    with tile.TileContext(nc) as tc:
        with ExitStack() as ctx:
            persist = ctx.enter_context(tc.tile_pool(name="persist", bufs=1))
            fin_pool = ctx.enter_context(tc.tile_pool(name="finpool", bufs=6))

            qkT_sb = persist.tile([P, seq], f16, tag="qkT")
            cqkT_sb = persist.tile([P, seq], f16, tag="cqkT")
            v_sb = persist.tile([P, NT, hpc * vw], f16, tag="v")
            cv_sb = persist.tile([P, NT, hpc * vw], f16, tag="cv")
            wout_sb = persist.tile([P, dim], f16, tag="wout")
            wcout_sb = persist.tile([P, dim], f16, tag="wcout")
            outmT_sb = persist.tile([P, seq], f16, tag="outmT")
            ctxmT_sb = persist.tile([P, seq], f16, tag="ctxmT")
            f32r = mybir.dt.float32r
            ones_f = persist.tile([1, dh], f32, tag="onesf", name="ones_f")
            nc.vector.memset(ones_f, 1.0)
            ones_r = persist.tile([1, dh], f32r, tag="ones", name="ones_r")
            with nc.allow_low_precision(reason="ones constant, exact in f32r"):
                nc.vector.tensor_copy(ones_r, ones_f)

            nc.sync.dma_start(wout_sb, wout)
            nc.sync.dma_start(wcout_sb, wcout)

            # ---- load x/context + weights, compute projections, then release
            with tc.tile_pool(name="xcpool", bufs=1) as xc_pool, \
                 tc.tile_pool(name="psproj", bufs=8, space="PSUM") as ps_proj:
                w_sbs = {}
                for name, ap_ in (("wqk", wqk), ("wv", wv), ("wcqk", wcqk),
                                  ("wcv", wcv)):
                    t = xc_pool.tile([P, KT, fpc], f16, tag=name)
                    nc.sync.dma_start(t, ap_.rearrange("(kt p) f -> p kt f", p=P))
                    w_sbs[name] = t
                xT_sb = xc_pool.tile([P, KT, seq], f16, tag="xT")
                cT_sb = xc_pool.tile([P, KT, seq], f16, tag="cT")
                xT_v = xT.rearrange("(kt p) i -> p kt i", p=P)
                cT_v = cT.rearrange("(kt p) i -> p kt i", p=P)
                for kt in range(KT):
                    nc.sync.dma_start(xT_sb[:, kt], xT_v[:, kt])
                    nc.sync.dma_start(cT_sb[:, kt], cT_v[:, kt])

                from concourse.masks import make_identity
                ident = persist.tile([P, P], f16, tag="ident")
                make_identity(nc, ident)
                for h in range(hpc):
                    nc.vector.memset(v_sb[:, :, h * vw + dh], 1.0)
                    nc.vector.memset(cv_sb[:, :, h * vw + dh], 1.0)
                vT_tmps = {}
                vT_tmps["wv"] = persist.tile([P, seq], f16, tag="vT_wv", name="vT_wv")
                vT_tmps["wcv"] = persist.tile([P, seq], f16, tag="vT_wcv", name="vT_wcv")
                # projections: 4 tensors x 4 chunks; two chunk-group passes,
                # 8 psum accumulators live per pass, ktile-major so matmuls
                # chase the input DMAs
                projs = ((xT_sb, "wqk", qkT_sb), (cT_sb, "wcqk", cqkT_sb),
                         (xT_sb, "wv", vT_tmps["wv"]), (cT_sb, "wcv", vT_tmps["wcv"]))
                NGR = 2 if NCH >= 2 else 1          # chunk groups
                CPG = NCH // NGR                    # chunks per group
                for cg in range(NGR):
                    tiles = {}
                    for pi in range(4):
                        for cc in range(CPG):
                            tiles[(pi, cc)] = ps_proj.tile(
                                [P, 512], f32, tag="pp",
                                name=f"pp_{cg}_{pi}_{cc}")
                    for kt in range(KT):
                        for pi, (src_sb, wname, dst) in enumerate(projs):
                            for cc in range(CPG):
                                icx = cg * CPG + cc
                                nc.tensor.matmul(
                                    tiles[(pi, cc)], w_sbs[wname][:, kt],
                                    src_sb[:, kt, _ts(icx, 512)],
                                    start=(kt == 0), stop=(kt == KT - 1))
                    for pi, (src_sb, wname, dst) in enumerate(projs):
                        for cc in range(CPG):
                            icx = cg * CPG + cc
                            nc.vector.tensor_copy(dst[:, _ts(icx, 512)],
                                                  tiles[(pi, cc)])
            ps_pool = ctx.enter_context(
                tc.tile_pool(name="pspool", bufs=2, space="PSUM"))
            ps_acc = ctx.enter_context(
                tc.tile_pool(name="psacc", bufs=2, space="PSUM"))

            # ---- per-head attention (software-pipelined emission) ----
            # Phases = (head, j-half). Each phase emits sim+exp+transpose for
            # 16 i-tiles; H/G accumulation and normalization work from earlier
            # phases is sprinkled between iterations so PE work rides under
            # the ACT-bound exp stream.
            from collections import deque

            e_pool = ctx.enter_context(tc.tile_pool(name="epool", bufs=2))
            et_pool = ctx.enter_context(tc.tile_pool(name="etpool", bufs=1))
            hg_pool = ctx.enter_context(tc.tile_pool(name="hgpool", bufs=2))
            norm_pool = ctx.enter_context(tc.tile_pool(name="normpool", bufs=2))
            JPH = NT // N2CH           # j-tiles per half (8)

            eT_tiles = {}

            def vcv_transpose_work():
                for wname, dst in (("wv", v_sb), ("wcv", cv_sb)):
                    vT_tmp = vT_tmps[wname]
                    for ibg in range(NT // 4):
                        pst = ps_pool.tile([P, 1024], f32, tag="ps")
                        pst16 = pst.bitcast(f16)
                        for k in range(4):
                            nc.tensor.transpose(pst16[:, _ts(k, P)],
                                                vT_tmp[:, _ts(ibg * 4 + k, P)],
                                                ident)
                            yield
                        pstv = pst16[:, :4 * P].rearrange("p (k f) -> p k f", k=4)
                        for h in range(hpc):
                            nc.vector.tensor_copy(
                                dst[:, ibg * 4:(ibg + 1) * 4, h * vw:h * vw + dh],
                                pstv[:, :, h * dh:(h + 1) * dh])
                        yield

            def h_work(h, half, E_half):
                """Accumulate H^T chunks of this (head, j-half) + ctx norm."""
                hs = slice(h * dh, (h + 1) * dh)
                va = slice(h * vw, h * vw + vw)
                psH = ps_acc.tile([vw, J2], f32, tag="acc")
                hT = hg_pool.tile([vw, J2], f16, tag="ht")
                rcs_r = norm_pool.tile([1, J2], f32r, tag="rc", name="rcs_r")
                for jcc in range(HPT):
                    jsl_l = _ts(jcc, 512)
                    for it in range(NT):
                        nc.tensor.matmul(psH[:, jsl_l], v_sb[:, it, va],
                                         E_half[:, it, jsl_l],
                                         start=(it == 0), stop=(it == NT - 1))
                        yield
                    nc.scalar.copy(hT[:, jsl_l], psH[:, jsl_l])
                    with nc.allow_low_precision(reason="softmax sums O(2e3); f32r rounding is ~1e-7 rel"):
                        nc.vector.reciprocal(rcs_r[:, jsl_l], hT[dh:dh + 1, jsl_l])
                    jsl_g = _ts(half * HPT + jcc, 512)
                    bc = ps_pool.tile([P, 1024], f32, tag="ps")
                    nc.tensor.matmul(bc[:dh, :512], ones_r, rcs_r[:, jsl_l],
                                     start=True, stop=True)
                    nc.vector.tensor_mul(ctxmT_sb[hs, jsl_g], hT[0:dh, jsl_l],
                                         bc[:dh, :512])
                    yield

            def g_work(h):
                """Accumulate G^T for head h from eT + out-side norm."""
                hs = slice(h * dh, (h + 1) * dh)
                va = slice(h * vw, h * vw + vw)
                eT_h = eT_tiles[h]
                for ihalf in range(N2CH):
                    psG = ps_acc.tile([vw, J2], f32, tag="acc")
                    gT = hg_pool.tile([vw, J2], f16, tag="gt")
                    rrs_r = norm_pool.tile([1, J2], f32r, tag="rr", name="rrs_r")
                    for icc in range(HPT):
                        isl_l = _ts(icc, 512)
                        isl_g = _ts(ihalf * HPT + icc, 512)
                        for jt in range(NT):
                            nc.tensor.matmul(psG[:, isl_l], cv_sb[:, jt, va],
                                             eT_h[:, jt, isl_g],
                                             start=(jt == 0), stop=(jt == NT - 1))
                            yield
                        nc.vector.tensor_copy(gT[:, isl_l], psG[:, isl_l])
                        with nc.allow_low_precision(reason="softmax sums O(2e3); f32r rounding is ~1e-7 rel"):
                            nc.vector.reciprocal(rrs_r[:, isl_l], gT[dh:dh + 1, isl_l])
                        bc2 = ps_pool.tile([P, 1024], f32, tag="ps")
                        nc.tensor.matmul(bc2[:dh, :512], ones_r, rrs_r[:, isl_l],
                                         start=True, stop=True)
                        nc.vector.tensor_mul(outmT_sb[hs, isl_g], gT[0:dh, isl_l],
                                             bc2[:dh, :512])
                        yield

            pending = deque()
            pending.append(vcv_transpose_work())

            def sprinkle(n):
                done = 0
                while pending and done < n:
                    try:
                        next(pending[0])
                        done += 1
                    except StopIteration:
                        pending.popleft()

            phases = [(h, half) for h in range(hpc) for half in range(N2CH)]
            for h, half in phases:
                hs = slice(h * dh, (h + 1) * dh)
                E_half = e_pool.tile([P, NT, J2], f16, tag="e")
                if half == 0:
                    eT_tiles[h] = et_pool.tile([P, NT, seq], f16, tag="et", name=f"eT_{h}")
                for it in range(NT):
                    ps = ps_pool.tile([P, 1024], f32, tag="ps")
                    for hlf in range(HPT):
                        js = _ts(half * HPT + hlf, 512)
                        nc.tensor.matmul(ps[:, _ts(hlf, 512)],
                                         qkT_sb[hs, _ts(it, P)],
                                         cqkT_sb[hs, js],
                                         start=True, stop=True)
                    nc.scalar.activation(E_half[:, it, :], ps[:, :J2],
                                         Exp, scale=SCALE)
                    sprinkle(6)
                if stage not in ('e0',):
                    # DMA xbar transposes emitted as one contiguous block per
                    # j-half: they fire as a back-to-back burst (the fast
                    # regime) instead of interleaving with other traffic
                    for it in range(NT):
                        nc.sync.dma_start_transpose(
                            eT_tiles[h][:, _ts(half, JPH), _ts(it, P)],
                            E_half[:, it, :])
                if stage not in ('e0', 'e'):
                    pending.append(h_work(h, half, E_half))
                if half == N2CH - 1 and stage in ('full', 'gh'):
                    pending.append(g_work(h))
                # sequential mode: drain phase work here (coarse-grained sync)
                if SEQ_MODE:
                    while pending:
                        sprinkle(1 << 30)

            def final_work(si, mT, w_sb, odram):
                for ib in range(NT):
                    pso = ps_pool.tile([P, 1024], f32, tag="ps")
                    for oc in range(OCH):
                        nc.tensor.matmul(pso[:, _ts(oc, 512)], mT[:, _ts(ib, P)],
                                         w_sb[:, _ts(oc, 512)],
                                         start=True, stop=True)
                        yield
                    osb = fin_pool.tile([P, dim], f16, tag="osb")
                    nc.scalar.copy(osb, pso[:, :dim])
                    nc.sync.dma_start(odram[:, ib, :], osb)
                    yield

            out_view = out_p.rearrange("(ib p) o -> p ib o", p=P)
            ctx_view = ctx_p.rearrange("(ib p) o -> p ib o", p=P)
            if stage != 'full':
                while pending:
                    try:
                        next(pending[0])
                    except StopIteration:
                        pending.popleft()
                dummy = fin_pool.tile([P, dim], f16, tag="osb", name="dummy")
                nc.vector.memset(outmT_sb, 0.0)
                nc.vector.memset(ctxmT_sb, 0.0)
                nc.vector.memset(dummy, 0.0)
                nc.sync.dma_start(out_view[:, 0, :], dummy)
                nc.sync.dma_start(ctx_view[:, 0, :], dummy)
                _truncated = True
            else:
                _truncated = False
            # FIFO-drain everything except the last head's G stream
            while _truncated and pending:
                pending.popleft()
            while len(pending) > 1:
                try:
                    next(pending[0])
                except StopIteration:
                    pending.popleft()
            # interleave the remaining G stream with the ctx-side final
            pending.append(final_work(1, ctxmT_sb, wcout_sb, ctx_view))
            while pending:
                try:
                    next(pending[0])
                    pending.rotate(-1)
                except StopIteration:
                    pending.popleft()
            # out-side final strictly after outmT is complete
            for _ in final_work(0, outmT_sb, wout_sb, out_view):
                pass

    nc.compile()
    return nc


_NC_CACHE = {}


def _get_nc():
    if "nc" not in _NC_CACHE:
        _NC_CACHE["nc"] = build_bass()
    return _NC_CACHE["nc"]


def make_in_maps(x, context, W_qk, W_cqk, W_v, W_cv):
    f16 = np.float16
    xT = np.ascontiguousarray(np.asarray(x, np.float32)[0].T).astype(f16)
    cT = np.ascontiguousarray(np.asarray(context, np.float32)[0].T).astype(f16)
    in_maps = []
    for c in range(N_CORES):
        cs = _ts(c, FPC)
        in_maps.append({
            "xT": xT,
            "cT": cT,
            "wqk": np.ascontiguousarray(np.asarray(W_qk)[:, cs]).astype(f16),
            "wv": np.ascontiguousarray(np.asarray(W_v)[:, cs]).astype(f16),
            "wcqk": np.ascontiguousarray(np.asarray(W_cqk)[:, cs]).astype(f16),
            "wcv": np.ascontiguousarray(np.asarray(W_cv)[:, cs]).astype(f16),
        })
    return in_maps


def add_weight_slices(in_maps, W_out, W_cout):
    f16 = np.float16
    for c in range(N_CORES):
        rs = _ts(c, FPC)
        in_maps[c]["wout"] = np.ascontiguousarray(np.asarray(W_out)[rs, :]).astype(f16)
        in_maps[c]["wcout"] = np.ascontiguousarray(np.asarray(W_cout)[rs, :]).astype(f16)
    return in_maps


def kernel(x, context, W_qk, W_cqk, W_v, W_cv, W_out, b_out, W_cout, b_cout):
    from concourse.bass_utils import run_bass_kernel_spmd

    nc = _get_nc()
    in_maps = make_in_maps(x, context, W_qk, W_cqk, W_v, W_cv)
    add_weight_slices(in_maps, W_out, W_cout)

    res = run_bass_kernel_spmd(nc, in_maps, core_ids=list(range(N_CORES)))

    out = np.zeros((SEQ, DIM), np.float32)
    ctx_out = np.zeros((SEQ, DIM), np.float32)
    for r in res.results:
        out += r["out_p"].astype(np.float32)
        ctx_out += r["ctx_p"].astype(np.float32)
    out += np.asarray(b_out, np.float32)
    ctx_out += np.asarray(b_cout, np.float32)
    return (out[None], ctx_out[None])

